# revision 7
# baseline (speedup 1.0000x reference)
"""Trainium2 Bass kernel for a dense 3x3 Conv2D (NCHW, stride 1, pad 1) + bias.

Problem (hardcoded): x[32,128,56,56] f32, W[256,128,3,3] f32, b[256] f32
-> out[32,256,56,56] f32.

Strategy: data-parallel over the batch. Each of the 8 NeuronCores gets 4
images; W and b are replicated. On-core, the conv is expressed as 9
PSUM-accumulated matmuls (one per kernel tap): contraction dim = C_in = 128
(exactly the PE partition dim), stationary operand = W[kh,kw] as
[C_in, C_out_chunk(128)], moving operand = a shifted window of the
zero-padded input [C_in, 8 rows x 56 cols = 448].
"""

import numpy as np

import concourse.bacc as bacc
import concourse.bass as bass
import concourse.mybir as mybir
import concourse.tile as tile
from concourse.bass_utils import run_bass_kernel_spmd

N, C_IN, H, W_DIM = 32, 128, 56, 56
C_OUT, KK, PAD = 256, 3, 1
N_CORES = 8
NSH = N // N_CORES          # images per core
HP = H + 2 * PAD            # padded height/width = 58
NCH = C_OUT // 128          # c_out chunks of 128
RG = 8                      # output rows per PSUM tile
NG = H // RG                # row groups per image

DT = mybir.dt.float32r      # matmul operand dtype: fp32 storage, ~13-bit
                            # mantissa PE path at full (1 cyc/row) rate;
                            # measured rel err ~1.5e-4 vs 2.4e-3 for bf16

TRACE = False               # test.py flips this for profiling
LAST_RESULT = None          # BassKernelResults of the last run


def _build():
    nc = bacc.Bacc(trn_type="TRN2")

    # x arrives host-padded: [NSH, C_IN, 58, 58] with zero borders. This keeps
    # the load a single fully-contiguous DMA per image and keeps the wait
    # count on the first matmul small (walrus caps sync waits per LDW).
    x_d = nc.dram_tensor("xp", [NSH, C_IN, HP, HP], DT, kind="ExternalInput")
    wt_d = nc.dram_tensor("Wt", [C_IN, KK, KK, NCH, 128], DT, kind="ExternalInput")
    b_d = nc.dram_tensor("b2", [128, NCH], mybir.dt.float32, kind="ExternalInput")
    out_d = nc.dram_tensor(
        "out", [NSH, C_OUT, H, W_DIM], mybir.dt.float32, kind="ExternalOutput"
    )

    with tile.TileContext(nc) as tc:
        with (
            tc.tile_pool(name="wpool", bufs=1) as wpool,
            tc.tile_pool(name="bpool", bufs=1) as bpool,
            tc.tile_pool(name="xpool", bufs=2) as xpool,
            tc.tile_pool(name="opool", bufs=4) as opool,
            tc.tile_pool(name="pspool", bufs=8, space=bass.MemorySpace.PSUM) as pspool,
        ):
            wt = wpool.tile([C_IN, KK, KK, NCH, 128], DT)
            nc.sync.dma_start(out=wt[:], in_=wt_d[:])
            bsb = bpool.tile([128, NCH], mybir.dt.float32)
            nc.sync.dma_start(out=bsb[:], in_=b_d[:])

            for n in range(NSH):
                xp = xpool.tile([C_IN, HP, HP], DT)
                nc.sync.dma_start(out=xp[:], in_=x_d[n])
                for ch in range(NCH):
                    for g in range(NG):
                        ps = pspool.tile([128, RG, W_DIM], mybir.dt.float32)
                        for kh in range(KK):
                            for kw in range(KK):
                                nc.tensor.matmul(
                                    ps[:],
                                    wt[:, kh, kw, ch, :],
                                    xp[:, g * RG + kh : g * RG + kh + RG, kw : kw + W_DIM],
                                    start=(kh == 0 and kw == 0),
                                    stop=(kh == KK - 1 and kw == KK - 1),
                                )
                        ot = opool.tile([128, RG, W_DIM], mybir.dt.float32)
                        # PSUM -> SBUF drain with per-partition bias broadcast.
                        # DVE, not ACT: ScalarE bulk copies are ~4x slower and
                        # became the kernel bottleneck (97.9% busy).
                        nc.vector.tensor_scalar_add(
                            ot[:], ps[:], bsb[:, ch : ch + 1]
                        )
                        nc.sync.dma_start(
                            out=out_d[n, ch * 128 : (ch + 1) * 128, g * RG : (g + 1) * RG, :],
                            in_=ot[:],
                        )
    nc.compile()
    return nc


_NC = None


def kernel(x, W, b):
    global _NC, LAST_RESULT
    if _NC is None:
        _NC = _build()

    np_dt = mybir.dt.np(DT)
    # Weight repack: [co, ci, kh, kw] -> [ci, kh, kw, chunk, co'] so the
    # stationary operand for each tap is a contiguous [C_in, 128] slab.
    wt = np.ascontiguousarray(
        W.reshape(NCH, 128, C_IN, KK, KK).transpose(2, 3, 4, 0, 1)
    ).astype(np_dt)
    b2 = np.ascontiguousarray(b.reshape(NCH, 128).T).astype(np.float32)

    xpad = np.zeros((N, C_IN, HP, HP), dtype=np_dt)
    xpad[:, :, PAD : PAD + H, PAD : PAD + W_DIM] = x
    in_maps = []
    for c in range(N_CORES):
        xs = np.ascontiguousarray(xpad[c * NSH : (c + 1) * NSH])
        in_maps.append({"xp": xs, "Wt": wt, "b2": b2})

    res = run_bass_kernel_spmd(
        _NC, in_maps, list(range(N_CORES)), trace=TRACE
    )
    LAST_RESULT = res
    out = np.concatenate([m["out"] for m in res.results], axis=0)
    return out.astype(np.float32)


# revision 8
# speedup vs baseline: 1.2978x; 1.2978x over previous
"""Trainium2 Bass kernel for a dense 3x3 Conv2D (NCHW, stride 1, pad 1) + bias.

Problem (hardcoded): x[32,128,56,56] f32, W[256,128,3,3] f32, b[256] f32
-> out[32,256,56,56] f32.

Strategy: data-parallel over the batch. Each of the 8 NeuronCores gets 4
images; W and b are replicated. On-core, the conv is expressed as 9
PSUM-accumulated matmuls (one per kernel tap): contraction dim = C_in = 128
(exactly the PE partition dim), stationary operand = W[kh,kw] as
[C_in, C_out_chunk(128)], moving operand = a shifted window of the
zero-padded input [C_in, 8 rows x 56 cols = 448].
"""

import numpy as np

import concourse.bacc as bacc
import concourse.bass as bass
import concourse.mybir as mybir
import concourse.tile as tile
from concourse.bass_utils import run_bass_kernel_spmd

N, C_IN, H, W_DIM = 32, 128, 56, 56
C_OUT, KK, PAD = 256, 3, 1
N_CORES = 8
NSH = N // N_CORES          # images per core
HP = H + 2 * PAD            # padded height/width = 58
NCH = C_OUT // 128          # c_out chunks of 128
RG = 8                      # output rows per PSUM tile
NG = H // RG                # row groups per image

DT = mybir.dt.float16       # matmul operand dtype. fp16: full PE rate AND
                            # FWL fast-weight-load (f32r loads weights at
                            # 230ns/MM > the 187ns stream, leaving PE
                            # LDWEIGHTS-bound). Measured matmul rel err:
                            # fp16 2.9e-4, f32r 1.5e-4, bf16 2.4e-3.

TRACE = False               # test.py flips this for profiling
LAST_RESULT = None          # BassKernelResults of the last run


def _build():
    nc = bacc.Bacc(trn_type="TRN2")

    # x arrives host-padded: [NSH, C_IN, 58, 58] with zero borders. This keeps
    # the load a single fully-contiguous DMA per image and keeps the wait
    # count on the first matmul small (walrus caps sync waits per LDW).
    x_d = nc.dram_tensor("xp", [NSH, C_IN, HP, HP], DT, kind="ExternalInput")
    wt_d = nc.dram_tensor("Wt", [C_IN, KK, KK, NCH, 128], DT, kind="ExternalInput")
    b_d = nc.dram_tensor("b2", [128, NCH], mybir.dt.float32, kind="ExternalInput")
    out_d = nc.dram_tensor(
        "out", [NSH, C_OUT, H, W_DIM], mybir.dt.float32, kind="ExternalOutput"
    )

    with tile.TileContext(nc) as tc:
        with (
            tc.tile_pool(name="wpool", bufs=1) as wpool,
            tc.tile_pool(name="bpool", bufs=1) as bpool,
            tc.tile_pool(name="xpool", bufs=2) as xpool,
            tc.tile_pool(name="opool", bufs=4) as opool,
            tc.tile_pool(name="pspool", bufs=8, space=bass.MemorySpace.PSUM) as pspool,
        ):
            wt = wpool.tile([C_IN, KK, KK, NCH, 128], DT)
            nc.sync.dma_start(out=wt[:], in_=wt_d[:])
            bsb = bpool.tile([128, NCH], mybir.dt.float32)
            nc.sync.dma_start(out=bsb[:], in_=b_d[:])

            for n in range(NSH):
                xp = xpool.tile([C_IN, HP, HP], DT)
                nc.sync.dma_start(out=xp[:], in_=x_d[n])
                for ch in range(NCH):
                    for g in range(NG):
                        ps = pspool.tile([128, RG, W_DIM], mybir.dt.float32)
                        for kh in range(KK):
                            for kw in range(KK):
                                nc.tensor.matmul(
                                    ps[:],
                                    wt[:, kh, kw, ch, :],
                                    xp[:, g * RG + kh : g * RG + kh + RG, kw : kw + W_DIM],
                                    start=(kh == 0 and kw == 0),
                                    stop=(kh == KK - 1 and kw == KK - 1),
                                )
                        ot = opool.tile([128, RG, W_DIM], mybir.dt.float32)
                        # PSUM -> SBUF drain with per-partition bias broadcast.
                        # DVE, not ACT: ScalarE bulk copies are ~4x slower and
                        # became the kernel bottleneck (97.9% busy).
                        nc.vector.tensor_scalar_add(
                            ot[:], ps[:], bsb[:, ch : ch + 1]
                        )
                        nc.sync.dma_start(
                            out=out_d[n, ch * 128 : (ch + 1) * 128, g * RG : (g + 1) * RG, :],
                            in_=ot[:],
                        )
    nc.compile()
    return nc


_NC = None


def kernel(x, W, b):
    global _NC, LAST_RESULT
    if _NC is None:
        _NC = _build()

    np_dt = mybir.dt.np(DT)
    # Weight repack: [co, ci, kh, kw] -> [ci, kh, kw, chunk, co'] so the
    # stationary operand for each tap is a contiguous [C_in, 128] slab.
    wt = np.ascontiguousarray(
        W.reshape(NCH, 128, C_IN, KK, KK).transpose(2, 3, 4, 0, 1)
    ).astype(np_dt)
    b2 = np.ascontiguousarray(b.reshape(NCH, 128).T).astype(np.float32)

    xpad = np.zeros((N, C_IN, HP, HP), dtype=np_dt)
    xpad[:, :, PAD : PAD + H, PAD : PAD + W_DIM] = x
    in_maps = []
    for c in range(N_CORES):
        xs = np.ascontiguousarray(xpad[c * NSH : (c + 1) * NSH])
        in_maps.append({"xp": xs, "Wt": wt, "b2": b2})

    res = run_bass_kernel_spmd(
        _NC, in_maps, list(range(N_CORES)), trace=TRACE
    )
    LAST_RESULT = res
    out = np.concatenate([m["out"] for m in res.results], axis=0)
    return out.astype(np.float32)


# revision 9
# speedup vs baseline: 1.3155x; 1.0136x over previous
"""Trainium2 Bass kernel for a dense 3x3 Conv2D (NCHW, stride 1, pad 1) + bias.

Problem (hardcoded): x[32,128,56,56] f32, W[256,128,3,3] f32, b[256] f32
-> out[32,256,56,56] f32.

Strategy: data-parallel over the batch. Each of the 8 NeuronCores gets 4
images; W and b are replicated. On-core, the conv is expressed as 9
PSUM-accumulated matmuls (one per kernel tap): contraction dim = C_in = 128
(exactly the PE partition dim), stationary operand = W[kh,kw] as
[C_in, C_out_chunk(128)], moving operand = a shifted window of the
zero-padded input [C_in, 8 rows x 56 cols = 448].
"""

import numpy as np

import concourse.bacc as bacc
import concourse.bass as bass
import concourse.mybir as mybir
import concourse.tile as tile
from concourse.bass_utils import run_bass_kernel_spmd

N, C_IN, H, W_DIM = 32, 128, 56, 56
C_OUT, KK, PAD = 256, 3, 1
N_CORES = 8
NSH = N // N_CORES          # images per core
HP = H + 2 * PAD            # padded height/width = 58
NCH = C_OUT // 128          # c_out chunks of 128
RG = 8                      # output rows per PSUM tile
NG = H // RG                # row groups per image

DT = mybir.dt.float16       # matmul operand dtype. fp16: full PE rate AND
                            # FWL fast-weight-load (f32r loads weights at
                            # 230ns/MM > the 187ns stream, leaving PE
                            # LDWEIGHTS-bound). Measured matmul rel err:
                            # fp16 2.9e-4, f32r 1.5e-4, bf16 2.4e-3.

TRACE = False               # test.py flips this for profiling
LAST_RESULT = None          # BassKernelResults of the last run


def _build():
    nc = bacc.Bacc(trn_type="TRN2")

    # x arrives host-padded: [NSH, C_IN, 58, 58] with zero borders. This keeps
    # the load a single fully-contiguous DMA per image and keeps the wait
    # count on the first matmul small (walrus caps sync waits per LDW).
    x_d = nc.dram_tensor("xp", [NSH, C_IN, HP, HP], DT, kind="ExternalInput")
    wt_d = nc.dram_tensor("Wt", [C_IN, KK, KK, NCH, 128], DT, kind="ExternalInput")
    b_d = nc.dram_tensor("b2", [128, NCH], mybir.dt.float32, kind="ExternalInput")
    out_d = nc.dram_tensor(
        "out", [NSH, C_OUT, H, W_DIM], mybir.dt.float32, kind="ExternalOutput"
    )

    with tile.TileContext(nc) as tc:
        with (
            tc.tile_pool(name="wpool", bufs=1) as wpool,
            tc.tile_pool(name="bpool", bufs=1) as bpool,
            tc.tile_pool(name="xpool", bufs=2) as xpool,
            tc.tile_pool(name="opool", bufs=4) as opool,
            tc.tile_pool(name="pspool", bufs=8, space=bass.MemorySpace.PSUM) as pspool,
        ):
            # Weight load split by kh so the first taps land fast; the first
            # image's load is split into row chunks so g=0's matmuls can
            # start after ~1/5 of the image has arrived (Tile deps are
            # region-granular). Later images prefetch whole during compute.
            wt = wpool.tile([C_IN, KK, KK, NCH, 128], DT)
            for kh in range(KK):
                nc.sync.dma_start(out=wt[:, kh], in_=wt_d[:, kh])
            bsb = bpool.tile([128, NCH], mybir.dt.float32)
            nc.sync.dma_start(out=bsb[:], in_=b_d[:])

            for n in range(NSH):
                xp = xpool.tile([C_IN, HP, HP], DT)
                if n == 0:
                    for r0, r1 in [(0, 11), (11, 27), (27, 43), (43, HP)]:
                        nc.sync.dma_start(
                            out=xp[:, r0:r1, :], in_=x_d[n, :, r0:r1, :]
                        )
                else:
                    nc.sync.dma_start(out=xp[:], in_=x_d[n])
                for ch in range(NCH):
                    for g in range(NG):
                        ps = pspool.tile([128, RG, W_DIM], mybir.dt.float32)
                        for kh in range(KK):
                            for kw in range(KK):
                                nc.tensor.matmul(
                                    ps[:],
                                    wt[:, kh, kw, ch, :],
                                    xp[:, g * RG + kh : g * RG + kh + RG, kw : kw + W_DIM],
                                    start=(kh == 0 and kw == 0),
                                    stop=(kh == KK - 1 and kw == KK - 1),
                                )
                        ot = opool.tile([128, RG, W_DIM], mybir.dt.float32)
                        # PSUM -> SBUF drain with per-partition bias broadcast.
                        # DVE, not ACT: ScalarE bulk copies are ~4x slower and
                        # became the kernel bottleneck (97.9% busy).
                        nc.vector.tensor_scalar_add(
                            ot[:], ps[:], bsb[:, ch : ch + 1]
                        )
                        nc.sync.dma_start(
                            out=out_d[n, ch * 128 : (ch + 1) * 128, g * RG : (g + 1) * RG, :],
                            in_=ot[:],
                        )
    nc.compile()
    return nc


_NC = None


def kernel(x, W, b):
    global _NC, LAST_RESULT
    if _NC is None:
        _NC = _build()

    np_dt = mybir.dt.np(DT)
    # Weight repack: [co, ci, kh, kw] -> [ci, kh, kw, chunk, co'] so the
    # stationary operand for each tap is a contiguous [C_in, 128] slab.
    wt = np.ascontiguousarray(
        W.reshape(NCH, 128, C_IN, KK, KK).transpose(2, 3, 4, 0, 1)
    ).astype(np_dt)
    b2 = np.ascontiguousarray(b.reshape(NCH, 128).T).astype(np.float32)

    xpad = np.zeros((N, C_IN, HP, HP), dtype=np_dt)
    xpad[:, :, PAD : PAD + H, PAD : PAD + W_DIM] = x
    in_maps = []
    for c in range(N_CORES):
        xs = np.ascontiguousarray(xpad[c * NSH : (c + 1) * NSH])
        in_maps.append({"xp": xs, "Wt": wt, "b2": b2})

    res = run_bass_kernel_spmd(
        _NC, in_maps, list(range(N_CORES)), trace=TRACE
    )
    LAST_RESULT = res
    out = np.concatenate([m["out"] for m in res.results], axis=0)
    return out.astype(np.float32)


# revision 10
# speedup vs baseline: 1.3534x; 1.0288x over previous
"""Trainium2 Bass kernel for a dense 3x3 Conv2D (NCHW, stride 1, pad 1) + bias.

Problem (hardcoded): x[32,128,56,56] f32, W[256,128,3,3] f32, b[256] f32
-> out[32,256,56,56] f32.

Strategy: data-parallel over the batch. Each of the 8 NeuronCores gets 4
images; W and b are replicated. On-core, the conv is expressed as 9
PSUM-accumulated matmuls (one per kernel tap): contraction dim = C_in = 128
(exactly the PE partition dim), stationary operand = W[kh,kw] as
[C_in, C_out_chunk(128)], moving operand = a shifted window of the
zero-padded input [C_in, 8 rows x 56 cols = 448].
"""

import numpy as np

import concourse.bacc as bacc
import concourse.bass as bass
import concourse.mybir as mybir
import concourse.tile as tile
from concourse.bass_utils import run_bass_kernel_spmd

N, C_IN, H, W_DIM = 32, 128, 56, 56
C_OUT, KK, PAD = 256, 3, 1
N_CORES = 8
NSH = N // N_CORES          # images per core
HP = H + 2 * PAD            # padded height/width = 58
NCH = C_OUT // 128          # c_out chunks of 128
RG = 8                      # output rows per PSUM tile
NG = H // RG                # row groups per image

DT = mybir.dt.float16       # matmul operand dtype. fp16: full PE rate AND
                            # FWL fast-weight-load (f32r loads weights at
                            # 230ns/MM > the 187ns stream, leaving PE
                            # LDWEIGHTS-bound). Measured matmul rel err:
                            # fp16 2.9e-4, f32r 1.5e-4, bf16 2.4e-3.

TRACE = False               # test.py flips this for profiling
LAST_RESULT = None          # BassKernelResults of the last run


def _build():
    nc = bacc.Bacc(trn_type="TRN2")

    # x arrives host-padded: [NSH, C_IN, 58, 58] with zero borders. This keeps
    # the load a single fully-contiguous DMA per image and keeps the wait
    # count on the first matmul small (walrus caps sync waits per LDW).
    x_d = nc.dram_tensor("xp", [NSH, C_IN, HP, HP], DT, kind="ExternalInput")
    wt_d = nc.dram_tensor("Wt", [C_IN, KK, KK, NCH, 128], DT, kind="ExternalInput")
    b_d = nc.dram_tensor("b2", [128, NCH], mybir.dt.float32, kind="ExternalInput")
    out_d = nc.dram_tensor(
        "out", [NSH, C_OUT, H, W_DIM], mybir.dt.float32, kind="ExternalOutput"
    )

    with tile.TileContext(nc) as tc:
        with (
            tc.tile_pool(name="wpool", bufs=1) as wpool,
            tc.tile_pool(name="bpool", bufs=1) as bpool,
            tc.tile_pool(name="xpool", bufs=2) as xpool,
            tc.tile_pool(name="opool", bufs=4) as opool,
            tc.tile_pool(name="pspool", bufs=8, space=bass.MemorySpace.PSUM) as pspool,
        ):
            # Head-latency critical: the g=0 matmuls need image-0 rows 0..10
            # and all 9 weight taps. Dispatch those pieces first (they fan
            # out across HWDGE queues and run concurrently); the rest of
            # image 0 streams in row chunks while g=0 computes. Tile deps
            # are region-granular, so each group waits only for its rows.
            wt = wpool.tile([C_IN, KK, KK, NCH, 128], DT)
            xp0 = xpool.tile([C_IN, HP, HP], DT, name="xp", tag="xp")
            nc.sync.dma_start(out=xp0[:, 0:11, :], in_=x_d[0, :, 0:11, :])
            for kh in range(KK):
                nc.sync.dma_start(out=wt[:, kh], in_=wt_d[:, kh])
            for r0, r1 in [(11, 27), (27, 43), (43, HP)]:
                nc.sync.dma_start(out=xp0[:, r0:r1, :], in_=x_d[0, :, r0:r1, :])
            bsb = bpool.tile([128, NCH], mybir.dt.float32)
            nc.sync.dma_start(out=bsb[:], in_=b_d[:])

            for n in range(NSH):
                if n == 0:
                    xp = xp0
                else:
                    xp = xpool.tile([C_IN, HP, HP], DT, name="xp", tag="xp")
                    nc.sync.dma_start(out=xp[:], in_=x_d[n])
                for ch in range(NCH):
                    for g in range(NG):
                        ps = pspool.tile([128, RG, W_DIM], mybir.dt.float32)
                        for kh in range(KK):
                            for kw in range(KK):
                                nc.tensor.matmul(
                                    ps[:],
                                    wt[:, kh, kw, ch, :],
                                    xp[:, g * RG + kh : g * RG + kh + RG, kw : kw + W_DIM],
                                    start=(kh == 0 and kw == 0),
                                    stop=(kh == KK - 1 and kw == KK - 1),
                                )
                        ot = opool.tile([128, RG, W_DIM], mybir.dt.float32)
                        # PSUM -> SBUF drain with per-partition bias broadcast.
                        # DVE, not ACT: ScalarE bulk copies are ~4x slower and
                        # became the kernel bottleneck (97.9% busy).
                        nc.vector.tensor_scalar_add(
                            ot[:], ps[:], bsb[:, ch : ch + 1]
                        )
                        nc.sync.dma_start(
                            out=out_d[n, ch * 128 : (ch + 1) * 128, g * RG : (g + 1) * RG, :],
                            in_=ot[:],
                        )
    nc.compile()
    return nc


_NC = None


def kernel(x, W, b):
    global _NC, LAST_RESULT
    if _NC is None:
        _NC = _build()

    np_dt = mybir.dt.np(DT)
    # Weight repack: [co, ci, kh, kw] -> [ci, kh, kw, chunk, co'] so the
    # stationary operand for each tap is a contiguous [C_in, 128] slab.
    wt = np.ascontiguousarray(
        W.reshape(NCH, 128, C_IN, KK, KK).transpose(2, 3, 4, 0, 1)
    ).astype(np_dt)
    b2 = np.ascontiguousarray(b.reshape(NCH, 128).T).astype(np.float32)

    xpad = np.zeros((N, C_IN, HP, HP), dtype=np_dt)
    xpad[:, :, PAD : PAD + H, PAD : PAD + W_DIM] = x
    in_maps = []
    for c in range(N_CORES):
        xs = np.ascontiguousarray(xpad[c * NSH : (c + 1) * NSH])
        in_maps.append({"xp": xs, "Wt": wt, "b2": b2})

    res = run_bass_kernel_spmd(
        _NC, in_maps, list(range(N_CORES)), trace=TRACE
    )
    LAST_RESULT = res
    out = np.concatenate([m["out"] for m in res.results], axis=0)
    return out.astype(np.float32)
